# revision 25
# baseline (speedup 1.0000x reference)
"""Distributed ALiBi causal attention for 8 TRN2 NeuronCores.

Sharding: core c = (b, hg) with b = c // 4 (batch), hg = c % 4 (group of 4
heads = 256 of the 1024 model dims).

The per-core kernel is one software pipeline over q-chunks of 512 tokens,
ascending:

    proj(kT, v, qT | chunk qc) -> attention(qc, head-pair 0/1)
        -> AllGather(unit) -> out-proj(chunk, transposed)

Causality means chunk qc's attention reads only kT/v chunks 0..qc, so the
(ACT-bound) exp stream of chunk qc overlaps the (PE-bound) projections of
chunk qc+1: both engines stay dense, the PE stays HAM-warm, and every
AllGather fires mid-pipeline instead of piling up at the end.

Attention runs in "scores transposed" form (scoresT[j, i] = k_j . q_i); the
ALiBi bias reduces to a per-key factor exp(-slope*j - ln 64) applied as a
V-row prescale (the per-query part cancels in the softmax ratio), and the
softmax denominators come free from an expb column appended to V.  Softmax
normalization: one DVE reciprocal of the denominator row, a contraction-1
matmul broadcasts it across partitions, two DVE multiplies produce the
fp16 gather payload — the partition shift for the odd head happens in the
(partition-agnostic) DMA to the collective's DRAM bounce buffer.

Out-projection is transposed (stationary Wo chunk, moving gathered heads,
N=512) and written as outT [DG, T]; the host transposes back.

Matmuls run in fp16 (10-bit mantissa = TF32-level accuracy at full PE
rate); accumulation is always fp32 in PSUM.
"""

import math
import os

import numpy as np

B = 2
T = 2048
C = 1024
H = 16
D = 64
N_CORES = 8
HG = 4          # head groups (cores per batch)
HL = 4          # heads per core
DG = HL * D     # 256 d-dims per core
CI = C // 128   # 8 contraction chunks of 128
TB = T // 128   # 16 row blocks of 128
QC = T // 512   # 4 q chunks of 512
VROW = D + 1    # v columns per head incl. expb (denominator) column
LN_SHIFT = float(np.log(64.0))
LAG = 2         # exp -> AV pipeline lag (in key blocks)

REPLICA_GROUPS = [[0, 1, 2, 3], [4, 5, 6, 7]]

_COMPILED = None
last_exec_time_ns = None
last_trace_path = None


def _alibi_slopes(n_heads: int) -> np.ndarray:
    def pow2_slopes(n):
        start = 2 ** (-(2 ** (-(math.log2(n) - 3))))
        return [start * start**i for i in range(n)]

    if math.log2(n_heads).is_integer():
        s = pow2_slopes(n_heads)
    else:
        c = 2 ** math.floor(math.log2(n_heads))
        s = pow2_slopes(c)
        s.extend(pow2_slopes(2 * c)[0::2][: n_heads - c])
    return np.array(s, dtype=np.float32)


def _build():
    import concourse.mybir as mybir
    import concourse.tile as tile
    from concourse.alu_op_type import AluOpType
    from concourse.bacc import Bacc
    from contextlib import ExitStack

    F32 = mybir.dt.float32
    F16 = mybir.dt.float16
    ACT = mybir.ActivationFunctionType

    nc = Bacc(None, target_bir_lowering=False, num_devices=N_CORES)

    xT_ext = nc.declare_dram_parameter("xT", [C, T], F16, isOutput=False)
    wq_ext = nc.declare_dram_parameter("wq", [C, DG], F16, isOutput=False)
    wk_ext = nc.declare_dram_parameter("wk", [C, DG], F16, isOutput=False)
    wv_ext = nc.declare_dram_parameter("wv", [C, HL * D], F16, isOutput=False)
    wo_ext = nc.declare_dram_parameter("wo", [C, DG], F16, isOutput=False)
    bq_ext = nc.declare_dram_parameter("bq2", [2, 128], F32, isOutput=False)
    bk_ext = nc.declare_dram_parameter("bk2", [2, 128], F32, isOutput=False)
    # expb[p, h*TB+tb] = exp(-slope_h * (128*tb+p) - ln 64): the ALiBi bias
    # as a per-key multiplicative prescale of V (incl. its denominator col).
    expb_ext = nc.declare_dram_parameter("expb", [128, HL * TB], F32, isOutput=False)
    tri_ext = nc.declare_dram_parameter("tri", [128, 128], F16, isOutput=False)
    ones_ext = nc.declare_dram_parameter("ones1", [1, 128], F32, isOutput=False)
    outT_ext = nc.declare_dram_parameter("outT", [DG, T], F32, isOutput=True)
    DBG = bool(os.environ.get("BASS_KERNEL_DEBUG"))
    if DBG:
        dbgg_ext = nc.declare_dram_parameter("dbgg", [128, CI * T], F16, isOutput=True)
        dbgqk_ext = nc.declare_dram_parameter("dbgqk", [128, 4 * T], F16, isOutput=True)
        dbgv_ext = nc.declare_dram_parameter(
            "dbgv", [128, TB * HL * VROW], F16, isOutput=True
        )
        dbga_ext = nc.declare_dram_parameter("dbga", [128, 2 * T], F16, isOutput=True)
        dbgr_ext = nc.declare_dram_parameter("dbgr", [1, 8 * 1024], F32, isOutput=True)
        dbge_ext = nc.declare_dram_parameter("dbge", [128, 4 * 1024], F16, isOutput=True)
        dbgav_ext = nc.declare_dram_parameter("dbgav", [VROW, 8 * 1024], F32, isOutput=True)

    with tile.TileContext(nc) as tc, ExitStack() as ctx:
        persist = ctx.enter_context(tc.tile_pool(name="persist", bufs=1))
        wo_sb = persist.tile([128, CI * DG], F16)
        expb_sb = persist.tile([128, HL * TB], F32)
        bq_sb = persist.tile([128, 2], F32)
        bk_sb = persist.tile([128, 2], F32)
        tri_sb = persist.tile([128, 128], F16)
        ones_sb = persist.tile([1, 128], F32)

        big = ctx.enter_context(tc.tile_pool(name="big", bufs=1))
        xT_sb = big.tile([128, CI * T], F16)
        wq_sb = big.tile([128, CI * DG], F16)
        wk_sb = big.tile([128, CI * DG], F16)
        wv_sb = big.tile([128, CI * HL * D], F16)
        qT_sb = big.tile([128, 2 * T], F16)
        kT_sb = big.tile([128, 2 * T], F16)
        v_sb = big.tile([128, TB * HL * VROW], F16)
        g_sb = big.tile([128, CI * T], F16)

        # ------- input DMA: few fat calls (SWDGE issue overhead ~1us/call)
        xT_src = xT_ext.rearrange("(ci p) t -> p ci t", p=128)
        xT_dst = xT_sb[:, :].rearrange("p (ci t) -> p ci t", ci=CI)
        wk_src = wk_ext.rearrange("(ci p) j -> p ci j", p=128)
        wq_src = wq_ext.rearrange("(ci p) j -> p ci j", p=128)
        wv_src = wv_ext.rearrange("(ci p) j -> p ci j", p=128)
        wo_src = wo_ext.rearrange("(ci p) j -> p ci j", p=128)
        nc.sync.dma_start(
            wk_sb[:, :].rearrange("p (ci j) -> p ci j", ci=CI), wk_src
        )
        nc.sync.dma_start(xT_dst[:, :, 0:512], xT_src[:, :, 0:512])
        nc.sync.dma_start(
            wv_sb[:, :].rearrange("p (ci j) -> p ci j", ci=CI), wv_src
        )
        nc.sync.dma_start(
            wq_sb[:, :].rearrange("p (ci j) -> p ci j", ci=CI), wq_src
        )
        nc.sync.dma_start(expb_sb[:], expb_ext[:])
        nc.sync.dma_start(tri_sb[:], tri_ext[:])
        nc.sync.dma_start(ones_sb[:], ones_ext[:])
        nc.sync.dma_start(bq_sb[:], bq_ext.rearrange("d p -> p d"))
        nc.sync.dma_start(bk_sb[:], bk_ext.rearrange("d p -> p d"))
        nc.sync.dma_start(xT_dst[:, :, 512:1024], xT_src[:, :, 512:1024])
        nc.sync.dma_start(xT_dst[:, :, 1024:T], xT_src[:, :, 1024:T])
        nc.sync.dma_start(
            wo_sb[:, :].rearrange("p (ci j) -> p ci j", ci=CI), wo_src
        )

        # denominator columns of v: expb itself, strided across row blocks
        for h in range(HL):
            nc.vector.tensor_copy(
                v_sb[:, :].rearrange("p (t x) -> p t x", t=TB)[
                    :, :, h * VROW + D : h * VROW + D + 1
                ],
                expb_sb[:, h * TB : (h + 1) * TB].rearrange(
                    "p (t x) -> p t x", t=TB
                ),
            )

        # ---------------- pools ------------------------------------------
        # PSUM bank budget (8): mm 2x1 + qk 2x2 + av 1x2 = 8.
        mm_ps = ctx.enter_context(tc.tile_pool(name="mm_ps", bufs=2, space="PSUM"))
        qk_ps = ctx.enter_context(tc.tile_pool(name="qk_ps", bufs=2, space="PSUM"))
        av_ps = ctx.enter_context(tc.tile_pool(name="av_ps", bufs=1, space="PSUM"))
        expp = ctx.enter_context(tc.tile_pool(name="expp", bufs=4))
        nrm = ctx.enter_context(tc.tile_pool(name="nrm", bufs=2))
        attp = ctx.enter_context(tc.tile_pool(name="attp", bufs=2))
        outp = ctx.enter_context(tc.tile_pool(name="outp", bufs=2))
        dram = ctx.enter_context(tc.tile_pool(name="dram", bufs=1, space="DRAM"))

        # tiny warm-up AllGather: absorbs inter-core start skew so the first
        # real gather's barrier wait does not stall the pipeline
        warm_in = dram.tile([1, 8], F16, tag="warm_in", name="warm_in")
        warm_out = dram.tile([HG, 8], F16, tag="warm_out", name="warm_out")
        nc.sync.dma_start(warm_in[:], tri_ext[0:1, 0:8])
        nc.gpsimd.collective_compute(
            "AllGather",
            mybir.AluOpType.bypass,
            replica_groups=REPLICA_GROUPS,
            ins=[warm_in[:].opt()],
            outs=[warm_out[:].opt()],
        )

        wo_queue: list[int] = []
        wo_stage2: list[int] = []
        wo_stage: list[int] = []
        state = {"pending_norm": None}

        def proj_qk(w_sb_, t_sb, b_sb, qc, nm):
            for db in (0, 1):
                ps = mm_ps.tile([128, 512], F32, tag="mm", name=f"p{nm}{qc}{db}")
                for ci in range(CI):
                    nc.tensor.matmul(
                        ps[:],
                        w_sb_[:, ci * DG + db * 128 : ci * DG + db * 128 + 128],
                        xT_sb[:, ci * T + qc * 512 : ci * T + qc * 512 + 512],
                        start=(ci == 0),
                        stop=(ci == CI - 1),
                    )
                nc.vector.tensor_scalar_add(
                    t_sb[:, db * T + qc * 512 : db * T + qc * 512 + 512],
                    ps[:],
                    b_sb[:, db : db + 1],
                )

        def proj_v(qc):
            for tb in range(4 * qc, 4 * qc + 4):
                ps = mm_ps.tile([128, 512], F32, tag="mm", name=f"pv{tb}")
                for ci in range(CI):
                    nc.tensor.matmul(
                        ps[:, 0 : HL * D],
                        xT_sb[:, ci * T + tb * 128 : ci * T + tb * 128 + 128],
                        wv_sb[:, ci * HL * D : (ci + 1) * HL * D],
                        start=(ci == 0),
                        stop=(ci == CI - 1),
                    )
                for h in range(HL):
                    nc.vector.tensor_scalar_mul(
                        v_sb[
                            :,
                            tb * HL * VROW + h * VROW : tb * HL * VROW
                            + h * VROW
                            + D,
                        ],
                        ps[:, h * D : (h + 1) * D],
                        expb_sb[:, h * TB + tb : h * TB + tb + 1],
                    )

        def emit_wo_chunk(qcw):
            # transposed out-proj: outT[co, t] for chunk qcw's 512 tokens
            for ch in (0, 1):
                wp = mm_ps.tile([128, 512], F32, tag="mm", name=f"wp{qcw}{ch}")
                for ci in range(CI):
                    nc.tensor.matmul(
                        wp[:],
                        wo_sb[:, ci * DG + ch * 128 : ci * DG + ch * 128 + 128],
                        g_sb[:, ci * T + qcw * 512 : ci * T + qcw * 512 + 512],
                        start=(ci == 0),
                        stop=(ci == CI - 1),
                    )
                ot = outp.tile([128, 512], F32, tag="out", name=f"ot{qcw}{ch}")
                nc.vector.tensor_copy(ot[:], wp[:])
                nc.sync.dma_start(
                    outT_ext[ch * 128 : (ch + 1) * 128, qcw * 512 : (qcw + 1) * 512],
                    ot[:],
                )

        def make_norm(qc, hp, av, rr):
            # deferred into the next unit's kb loop so the chain never
            # head-of-line-blocks the PE queue (recip already ran at unit end)
            def run():
                bs0 = attp.tile([64, 512], F32, tag="bs0", name=f"bs0_{qc}{hp}")
                bs1 = attp.tile([64, 512], F32, tag="bs1", name=f"bs1_{qc}{hp}")
                nc.gpsimd.partition_broadcast(bs0[:], rr[0:1, 0:512])
                nc.gpsimd.partition_broadcast(bs1[:], rr[0:1, 512:1024])
                at0 = attp.tile([64, 512], F16, tag="at0", name=f"at0_{qc}{hp}")
                at1 = attp.tile([64, 512], F16, tag="at1", name=f"at1_{qc}{hp}")
                nc.vector.tensor_tensor(
                    at0[:], av[0:D, 0:512], bs0[:], AluOpType.mult
                )
                nc.vector.tensor_tensor(
                    at1[:], av[0:D, 512:1024], bs1[:], AluOpType.mult
                )
                if qc < QC - 1:
                    # chunks 0..2: one AllGather per chunk (both head-pairs)
                    if hp == 0:
                        state[("ad", qc)] = dram.tile(
                            [256, 512], F16, tag=f"ad{qc}", name=f"ad{qc}"
                        )
                    attn_dram = state[("ad", qc)]
                    ofs = hp * 128
                else:
                    # last chunk: per-unit gathers so the tail only waits on
                    # the final head-pair's small gather
                    attn_dram = dram.tile(
                        [128, 512], F16, tag=f"ad{qc}_{hp}", name=f"ad{qc}_{hp}"
                    )
                    ofs = 0
                # partition shift for the odd head happens here for free
                nc.sync.dma_start(attn_dram[ofs : ofs + 64, :], at0[:])
                nc.sync.dma_start(attn_dram[ofs + 64 : ofs + 128, :], at1[:])
                if DBG:
                    sl_ = slice(hp * T + qc * 512, hp * T + qc * 512 + 512)
                    nc.sync.dma_start(dbga_ext[0:64, sl_], at0[:])
                    nc.sync.dma_start(dbga_ext[64:128, sl_], at1[:])
                    u_ = 2 * qc + hp
                    nc.sync.dma_start(
                        dbgr_ext[0:1, u_ * 1024 : (u_ + 1) * 1024], rr[:]
                    )
                if qc < QC - 1:
                    if hp == 1:
                        gathered = dram.tile(
                            [HG * 256, 512], F16, tag=f"gd{qc}", name=f"gd{qc}"
                        )
                        nc.gpsimd.collective_compute(
                            "AllGather",
                            mybir.AluOpType.bypass,
                            replica_groups=REPLICA_GROUPS,
                            ins=[attn_dram[:].opt()],
                            outs=[gathered[:].opt()],
                        )
                        nc.sync.dma_start(
                            g_sb[:, :].rearrange("p (ci t) -> p ci t", ci=CI)[
                                :, :, qc * 512 : qc * 512 + 512
                            ],
                            gathered[:].rearrange("(ci p) t -> p ci t", p=128),
                        )
                        wo_stage.append(qc)
                else:
                    gathered = dram.tile(
                        [HG * 128, 512], F16, tag=f"gd{qc}_{hp}", name=f"gd{qc}_{hp}"
                    )
                    nc.gpsimd.collective_compute(
                        "AllGather",
                        mybir.AluOpType.bypass,
                        replica_groups=REPLICA_GROUPS,
                        ins=[attn_dram[:].opt()],
                        outs=[gathered[:].opt()],
                    )
                    nc.sync.dma_start(
                        g_sb[:, :].rearrange("p (g c) -> p g c", g=HG)[
                            :, :, hp * T + qc * 512 : hp * T + qc * 512 + 512
                        ],
                        gathered[:].rearrange("(g p) t -> p g t", p=128),
                    )
                    if hp == 1:
                        wo_stage.append(qc)

            return run

        def attention_unit(qc, hp):
            nkb = 4 * (qc + 1)
            h0, h1 = 2 * hp, 2 * hp + 1
            q0 = qT_sb[0:64, hp * T + qc * 512 : hp * T + qc * 512 + 512]
            q1 = qT_sb[64:128, hp * T + qc * 512 : hp * T + qc * 512 + 512]
            av = av_ps.tile([VROW, 1024], F32, tag="av", name="av")
            ets = {}

            def emit_qk(kb):
                r = kb - 4 * qc
                c0 = 128 * r if r > 0 else 0
                # both heads' scoresT in one 2-bank tile; the two matmuls
                # ride different PE row groups and run concurrently
                qkp = qk_ps.tile([128, 1024], F32, tag="qk", name="qkp")
                nc.tensor.matmul(
                    qkp[:, c0:512],
                    kT_sb[0:64, hp * T + kb * 128 : hp * T + kb * 128 + 128],
                    q0[:, c0:512],
                    start=True,
                    stop=True,
                )
                nc.tensor.matmul(
                    qkp[:, 512 + c0 : 1024],
                    kT_sb[64:128, hp * T + kb * 128 : hp * T + kb * 128 + 128],
                    q1[:, c0:512],
                    start=True,
                    stop=True,
                )
                et = expp.tile([128, 1024], F16, tag="exp", name="e")
                ets[kb] = et
                # single bias-free exp over both heads (ALiBi lives in the V
                # prescale); the [0:c0) strips are stale-but-finite junk that
                # AV never reads
                nc.scalar.activation(
                    et[:], qkp[:], ACT.Exp, scale=float(D) ** -0.5
                )
                if r >= 0:
                    for eoff in (0, 512):
                        nc.vector.tensor_tensor(
                            et[:, eoff + c0 : eoff + c0 + 128],
                            et[:, eoff + c0 : eoff + c0 + 128],
                            tri_sb[:],
                            AluOpType.mult,
                        )
                if DBG and qc == 0 and hp == 0:
                    nc.sync.dma_start(
                        dbge_ext[:, kb * 1024 : (kb + 1) * 1024], et[:]
                    )

            def emit_av(kb):
                r = kb - 4 * qc
                c0 = 128 * r if r > 0 else 0
                et = ets.pop(kb)
                for h, eoff in ((h0, 0), (h1, 512)):
                    nc.tensor.matmul(
                        av[:, eoff + c0 : eoff + 512],
                        v_sb[
                            :,
                            kb * HL * VROW + h * VROW : kb * HL * VROW
                            + (h + 1) * VROW,
                        ],
                        et[:, eoff + c0 : eoff + 512],
                        start=(kb == 0),
                        stop=(kb == nkb - 1),
                    )

            # kb processed in pairs: 4 QK matmuls (64x128 tiling mode) then
            # 4 AV matmuls (128x128 mode) per step, halving PE mode switches
            for kb2 in range(0, nkb + LAG, 2):
                if kb2 == 2 and state["pending_norm"] is not None:
                    state["pending_norm"]()
                    state["pending_norm"] = None
                for kb in (kb2, kb2 + 1):
                    if kb < nkb:
                        emit_qk(kb)
                for kb in (kb2, kb2 + 1):
                    if LAG <= kb < nkb + LAG:
                        emit_av(kb - LAG)

            # reciprocal of both heads' denominator rows right away (DVE is
            # free here); the rest of the norm chain is deferred
            dn = nrm.tile([VROW, 1024], F32, tag="dn", name=f"dn{qc}{hp}")
            nc.vector.tensor_copy(dn[D : D + 1, :], av[D : D + 1, 0:1024])
            rri = nrm.tile([1, 1024], F32, tag="rri", name=f"rri{qc}{hp}")
            nc.sync.dma_start(rri[:], dn[D : D + 1, :])
            rr = nrm.tile([1, 1024], F32, tag="rr", name=f"rr{qc}{hp}")
            nc.vector.reciprocal_approx_fast(rr[:], rri[:])
            if DBG:
                u_ = 2 * qc + hp
                avc = nrm.tile([VROW, 1024], F32, tag="avc", name=f"avc{qc}{hp}")
                nc.vector.tensor_copy(avc[:], av[:])
                nc.sync.dma_start(
                    dbgav_ext[:, u_ * 1024 : (u_ + 1) * 1024], avc[:]
                )

            # out-projections whose gathers fired >= 3 units ago
            while wo_queue:
                emit_wo_chunk(wo_queue.pop(0))
            wo_queue.extend(wo_stage2)
            del wo_stage2[:]
            wo_stage2.extend(wo_stage)
            del wo_stage[:]

            state["pending_norm"] = make_norm(qc, hp, av, rr)

        # ---------------- the pipeline -----------------------------------
        for qc in range(QC):
            proj_qk(wk_sb, kT_sb, bk_sb, qc, "k")
            proj_v(qc)
            proj_qk(wq_sb, qT_sb, bq_sb, qc, "q")
            for hp in (0, 1):
                attention_unit(qc, hp)

        # fill the PE with the already-gathered chunks' out-projections
        # while the final unit's norm chain + last gather run
        for qcw in wo_queue + wo_stage2:
            emit_wo_chunk(qcw)
        state["pending_norm"]()
        for qcw in wo_stage:
            emit_wo_chunk(qcw)

        if DBG:
            nc.sync.dma_start(dbgg_ext[:], g_sb[:])
            nc.sync.dma_start(dbgqk_ext[:, 0 : 2 * T], qT_sb[:])
            nc.sync.dma_start(dbgqk_ext[:, 2 * T : 4 * T], kT_sb[:])
            nc.sync.dma_start(dbgv_ext[:], v_sb[:])

    nc.compile()
    return nc


def _get_compiled():
    global _COMPILED
    if _COMPILED is None:
        _COMPILED = _build()
    return _COMPILED


def _make_in_maps(x, Wq, bq, Wk, bk, Wv, bv, Wo, bo):
    slopes = _alibi_slopes(H)
    # tri[p, f] = 1 where key-offset p <= q-offset f (causal keep region)
    tri = np.triu(np.ones((128, 128), dtype=np.float16))
    ones1 = np.ones((1, 128), dtype=np.float32)
    in_maps = []
    for c in range(N_CORES):
        b, hg = divmod(c, HG)
        sl = slice(hg * DG, (hg + 1) * DG)
        # expb[p, h*TB+tb] = exp(-slope_h * j - ln 64) at key j = 128*tb + p
        expb = np.empty((128, HL * TB), dtype=np.float32)
        p = np.arange(128, dtype=np.float64)[:, None]
        for h in range(HL):
            s = float(slopes[hg * HL + h])
            tbs = np.arange(TB, dtype=np.float64)[None, :]
            expb[:, h * TB : (h + 1) * TB] = np.exp(
                -s * (128.0 * tbs + p) - LN_SHIFT
            ).astype(np.float32)
        in_maps.append(
            {
                "xT": np.ascontiguousarray(x[b].T).astype(np.float16),
                "wq": np.ascontiguousarray(Wq[:, sl]).astype(np.float16),
                "wk": np.ascontiguousarray(Wk[:, sl]).astype(np.float16),
                "wv": np.ascontiguousarray(Wv[:, sl]).astype(np.float16),
                "wo": np.ascontiguousarray(Wo[:, sl]).astype(np.float16),
                "bq2": np.ascontiguousarray(bq[sl].reshape(2, 128)).astype(np.float32),
                "bk2": np.ascontiguousarray(bk[sl].reshape(2, 128)).astype(np.float32),
                "expb": expb,
                "tri": tri,
                "ones1": ones1,
            }
        )
    return in_maps


def kernel(x, Wq, bq, Wk, bk, Wv, bv, Wo, bo):
    global last_exec_time_ns, last_trace_path
    x = np.asarray(x, dtype=np.float32)
    Wq = np.asarray(Wq, dtype=np.float32)
    bq = np.asarray(bq, dtype=np.float32)
    Wk = np.asarray(Wk, dtype=np.float32)
    bk = np.asarray(bk, dtype=np.float32)
    Wv = np.asarray(Wv, dtype=np.float32)
    bv = np.asarray(bv, dtype=np.float32)
    Wo = np.asarray(Wo, dtype=np.float32)
    bo = np.asarray(bo, dtype=np.float32)

    from concourse import bass_utils

    trace = bool(os.environ.get("BASS_KERNEL_TRACE"))
    kwargs = {}
    if trace:
        try:
            import sys
            import types

            import antenv

            if "antenv.axon_hooks" not in sys.modules:
                hooks = types.ModuleType("antenv.axon_hooks")
                _h = [None]
                hooks.set_axon_ntff_profile_hook = lambda fn: _h.__setitem__(0, fn)
                hooks.get_axon_ntff_profile_hook = lambda: _h[0]
                sys.modules["antenv.axon_hooks"] = hooks
                antenv.axon_hooks = hooks
                from trn_agent_boot.trn_boot import _ntff_profile_via_ctypes

                hooks.set_axon_ntff_profile_hook(
                    _ntff_profile_via_ctypes("/opt/axon/libaxon_pjrt.so")
                )
            bass_utils.upload_artifacts = lambda tmpdir: "local://" + str(tmpdir)
            kwargs = {"trace": True, "tmpdir": os.environ.get("BASS_KERNEL_TRACE_DIR")}
        except Exception as e:  # pragma: no cover
            print(f"trace setup failed ({e}); running untraced")
            trace = False

    nc = _get_compiled()
    in_maps = _make_in_maps(x, Wq, bq, Wk, bk, Wv, bv, Wo, bo)
    res = bass_utils.run_bass_kernel_spmd(
        nc, in_maps, core_ids=list(range(N_CORES)), **kwargs
    )
    if trace:
        last_exec_time_ns = res.exec_time_ns
        if res.instructions_and_trace is not None:
            last_trace_path = res.instructions_and_trace[1]

    # final-projection bias (incl. the v bias folded through Wo) on host
    bfin = bv @ Wo + bo  # [C]
    out = np.empty((B, T, C), dtype=np.float32)
    for c in range(N_CORES):
        b, hg = divmod(c, HG)
        sl = slice(hg * DG, (hg + 1) * DG)
        out[b, :, sl] = res.results[c]["outT"].T + bfin[sl]
    return out


# revision 27
# speedup vs baseline: 1.0471x; 1.0471x over previous
"""Distributed ALiBi causal attention for 8 TRN2 NeuronCores.

Sharding: core c = (b, hg) with b = c // 4 (batch), hg = c % 4 (group of 4
heads = 256 of the 1024 model dims).

The per-core kernel is one software pipeline over q-chunks of 512 tokens,
ascending:

    proj(kT, v, qT | chunk qc) -> attention(qc, head-pair 0/1)
        -> AllGather(unit) -> out-proj(chunk, transposed)

Causality means chunk qc's attention reads only kT/v chunks 0..qc, so the
(ACT-bound) exp stream of chunk qc overlaps the (PE-bound) projections of
chunk qc+1: both engines stay dense, the PE stays HAM-warm, and every
AllGather fires mid-pipeline instead of piling up at the end.

Attention runs in "scores transposed" form (scoresT[j, i] = k_j . q_i); the
ALiBi bias reduces to a per-key factor exp(-slope*j - ln 64) applied as a
V-row prescale (the per-query part cancels in the softmax ratio), and the
softmax denominators come free from an expb column appended to V.  Softmax
normalization: one DVE reciprocal of the denominator row, a contraction-1
matmul broadcasts it across partitions, two DVE multiplies produce the
fp16 gather payload — the partition shift for the odd head happens in the
(partition-agnostic) DMA to the collective's DRAM bounce buffer.

Out-projection is transposed (stationary Wo chunk, moving gathered heads,
N=512) and written as outT [DG, T]; the host transposes back.

Matmuls run in fp16 (10-bit mantissa = TF32-level accuracy at full PE
rate); accumulation is always fp32 in PSUM.
"""

import math
import os

import numpy as np

B = 2
T = 2048
C = 1024
H = 16
D = 64
N_CORES = 8
HG = 4          # head groups (cores per batch)
HL = 4          # heads per core
DG = HL * D     # 256 d-dims per core
CI = C // 128   # 8 contraction chunks of 128
TB = T // 128   # 16 row blocks of 128
QC = T // 512   # 4 q chunks of 512
VROW = D + 1    # v columns per head incl. expb (denominator) column
LN_SHIFT = float(np.log(64.0))
LAG = 2         # exp -> AV pipeline lag (in key blocks)

REPLICA_GROUPS = [[0, 1, 2, 3], [4, 5, 6, 7]]

_COMPILED = None
last_exec_time_ns = None
last_trace_path = None


def _alibi_slopes(n_heads: int) -> np.ndarray:
    def pow2_slopes(n):
        start = 2 ** (-(2 ** (-(math.log2(n) - 3))))
        return [start * start**i for i in range(n)]

    if math.log2(n_heads).is_integer():
        s = pow2_slopes(n_heads)
    else:
        c = 2 ** math.floor(math.log2(n_heads))
        s = pow2_slopes(c)
        s.extend(pow2_slopes(2 * c)[0::2][: n_heads - c])
    return np.array(s, dtype=np.float32)


def _build():
    import concourse.mybir as mybir
    import concourse.tile as tile
    from concourse.alu_op_type import AluOpType
    from concourse.bacc import Bacc
    from contextlib import ExitStack

    F32 = mybir.dt.float32
    F16 = mybir.dt.float16
    ACT = mybir.ActivationFunctionType

    nc = Bacc(None, target_bir_lowering=False, num_devices=N_CORES)

    xT_ext = nc.declare_dram_parameter("xT", [C, T], F16, isOutput=False)
    wq_ext = nc.declare_dram_parameter("wq", [C, DG], F16, isOutput=False)
    wk_ext = nc.declare_dram_parameter("wk", [C, DG], F16, isOutput=False)
    wv_ext = nc.declare_dram_parameter("wv", [C, HL * D], F16, isOutput=False)
    wo_ext = nc.declare_dram_parameter("wo", [C, DG], F16, isOutput=False)
    bq_ext = nc.declare_dram_parameter("bq2", [2, 128], F32, isOutput=False)
    bk_ext = nc.declare_dram_parameter("bk2", [2, 128], F32, isOutput=False)
    # expb[p, h*TB+tb] = exp(-slope_h * (128*tb+p) - ln 64): the ALiBi bias
    # as a per-key multiplicative prescale of V (incl. its denominator col).
    expb_ext = nc.declare_dram_parameter("expb", [128, HL * TB], F32, isOutput=False)
    tri_ext = nc.declare_dram_parameter("tri", [128, 128], F16, isOutput=False)
    ones_ext = nc.declare_dram_parameter("ones1", [1, 128], F32, isOutput=False)
    outT_ext = nc.declare_dram_parameter("outT", [DG, T], F32, isOutput=True)
    DBG = bool(os.environ.get("BASS_KERNEL_DEBUG"))
    if DBG:
        dbgg_ext = nc.declare_dram_parameter("dbgg", [128, CI * T], F16, isOutput=True)
        dbgqk_ext = nc.declare_dram_parameter("dbgqk", [128, 4 * T], F16, isOutput=True)
        dbgv_ext = nc.declare_dram_parameter(
            "dbgv", [128, TB * HL * VROW], F16, isOutput=True
        )
        dbga_ext = nc.declare_dram_parameter("dbga", [128, 2 * T], F16, isOutput=True)
        dbgr_ext = nc.declare_dram_parameter("dbgr", [1, 8 * 1024], F32, isOutput=True)
        dbge_ext = nc.declare_dram_parameter("dbge", [128, 4 * 1024], F16, isOutput=True)
        dbgav_ext = nc.declare_dram_parameter("dbgav", [VROW, 8 * 1024], F32, isOutput=True)

    with tile.TileContext(nc) as tc, ExitStack() as ctx:
        persist = ctx.enter_context(tc.tile_pool(name="persist", bufs=1))
        wo_sb = persist.tile([128, CI * DG], F16)
        expb_sb = persist.tile([128, HL * TB], F32)
        bq_sb = persist.tile([128, 2], F32)
        bk_sb = persist.tile([128, 2], F32)
        tri_sb = persist.tile([128, 128], F16)
        ones_sb = persist.tile([1, 128], F32)

        big = ctx.enter_context(tc.tile_pool(name="big", bufs=1))
        xT_sb = big.tile([128, CI * T], F16)
        wq_sb = big.tile([128, CI * DG], F16)
        wk_sb = big.tile([128, CI * DG], F16)
        wv_sb = big.tile([128, CI * HL * D], F16)
        qT_sb = big.tile([128, 2 * T], F16)
        kT_sb = big.tile([128, 2 * T], F16)
        v_sb = big.tile([128, TB * HL * VROW], F16)
        g_sb = big.tile([128, CI * T], F16)

        # ------- input DMA: few fat calls (SWDGE issue overhead ~1us/call)
        xT_src = xT_ext.rearrange("(ci p) t -> p ci t", p=128)
        xT_dst = xT_sb[:, :].rearrange("p (ci t) -> p ci t", ci=CI)
        wk_src = wk_ext.rearrange("(ci p) j -> p ci j", p=128)
        wq_src = wq_ext.rearrange("(ci p) j -> p ci j", p=128)
        wv_src = wv_ext.rearrange("(ci p) j -> p ci j", p=128)
        wo_src = wo_ext.rearrange("(ci p) j -> p ci j", p=128)
        nc.sync.dma_start(
            wk_sb[:, :].rearrange("p (ci j) -> p ci j", ci=CI), wk_src
        )
        nc.sync.dma_start(xT_dst[:, :, 0:512], xT_src[:, :, 0:512])
        nc.sync.dma_start(
            wv_sb[:, :].rearrange("p (ci j) -> p ci j", ci=CI), wv_src
        )
        nc.sync.dma_start(
            wq_sb[:, :].rearrange("p (ci j) -> p ci j", ci=CI), wq_src
        )
        nc.sync.dma_start(expb_sb[:], expb_ext[:])
        nc.sync.dma_start(tri_sb[:], tri_ext[:])
        nc.sync.dma_start(ones_sb[:], ones_ext[:])
        nc.sync.dma_start(bq_sb[:], bq_ext.rearrange("d p -> p d"))
        nc.sync.dma_start(bk_sb[:], bk_ext.rearrange("d p -> p d"))
        nc.sync.dma_start(xT_dst[:, :, 512:1024], xT_src[:, :, 512:1024])
        nc.sync.dma_start(xT_dst[:, :, 1024:T], xT_src[:, :, 1024:T])
        nc.sync.dma_start(
            wo_sb[:, :].rearrange("p (ci j) -> p ci j", ci=CI), wo_src
        )

        # denominator columns of v: expb itself, strided across row blocks
        for h in range(HL):
            nc.vector.tensor_copy(
                v_sb[:, :].rearrange("p (t x) -> p t x", t=TB)[
                    :, :, h * VROW + D : h * VROW + D + 1
                ],
                expb_sb[:, h * TB : (h + 1) * TB].rearrange(
                    "p (t x) -> p t x", t=TB
                ),
            )

        # ---------------- pools ------------------------------------------
        # PSUM bank budget (8): mm 2x1 + qk 2x2 + av 1x2 = 8.
        mm_ps = ctx.enter_context(tc.tile_pool(name="mm_ps", bufs=2, space="PSUM"))
        qk_ps = ctx.enter_context(tc.tile_pool(name="qk_ps", bufs=2, space="PSUM"))
        av_ps = ctx.enter_context(tc.tile_pool(name="av_ps", bufs=1, space="PSUM"))
        expp = ctx.enter_context(tc.tile_pool(name="expp", bufs=4))
        nrm = ctx.enter_context(tc.tile_pool(name="nrm", bufs=2))
        attp = ctx.enter_context(tc.tile_pool(name="attp", bufs=2))
        outp = ctx.enter_context(tc.tile_pool(name="outp", bufs=2))
        dram = ctx.enter_context(tc.tile_pool(name="dram", bufs=1, space="DRAM"))

        # tiny warm-up AllGather: absorbs inter-core start skew so the first
        # real gather's barrier wait does not stall the pipeline
        warm_in = dram.tile([1, 8], F16, tag="warm_in", name="warm_in")
        warm_out = dram.tile([HG, 8], F16, tag="warm_out", name="warm_out")
        nc.sync.dma_start(warm_in[:], tri_ext[0:1, 0:8])
        nc.gpsimd.collective_compute(
            "AllGather",
            mybir.AluOpType.bypass,
            replica_groups=REPLICA_GROUPS,
            ins=[warm_in[:].opt()],
            outs=[warm_out[:].opt()],
        )

        wo_queue: list[int] = []
        wo_stage2: list[int] = []
        wo_stage: list[int] = []
        state = {"pending_norm": None}

        def proj_qk(w_sb_, t_sb, b_sb, qc, nm):
            for db in (0, 1):
                ps = mm_ps.tile([128, 512], F32, tag="mm", name=f"p{nm}{qc}{db}")
                for ci in range(CI):
                    nc.tensor.matmul(
                        ps[:],
                        w_sb_[:, ci * DG + db * 128 : ci * DG + db * 128 + 128],
                        xT_sb[:, ci * T + qc * 512 : ci * T + qc * 512 + 512],
                        start=(ci == 0),
                        stop=(ci == CI - 1),
                    )
                nc.vector.tensor_scalar_add(
                    t_sb[:, db * T + qc * 512 : db * T + qc * 512 + 512],
                    ps[:],
                    b_sb[:, db : db + 1],
                )

        def proj_v(qc):
            for tb in range(4 * qc, 4 * qc + 4):
                ps = mm_ps.tile([128, 512], F32, tag="mm", name=f"pv{tb}")
                for ci in range(CI):
                    nc.tensor.matmul(
                        ps[:, 0 : HL * D],
                        xT_sb[:, ci * T + tb * 128 : ci * T + tb * 128 + 128],
                        wv_sb[:, ci * HL * D : (ci + 1) * HL * D],
                        start=(ci == 0),
                        stop=(ci == CI - 1),
                    )
                for h in range(HL):
                    nc.vector.tensor_scalar_mul(
                        v_sb[
                            :,
                            tb * HL * VROW + h * VROW : tb * HL * VROW
                            + h * VROW
                            + D,
                        ],
                        ps[:, h * D : (h + 1) * D],
                        expb_sb[:, h * TB + tb : h * TB + tb + 1],
                    )

        def emit_wo_chunk(qcw):
            # transposed out-proj: outT[co, t] for chunk qcw's 512 tokens
            for ch in (0, 1):
                wp = mm_ps.tile([128, 512], F32, tag="mm", name=f"wp{qcw}{ch}")
                for ci in range(CI):
                    nc.tensor.matmul(
                        wp[:],
                        wo_sb[:, ci * DG + ch * 128 : ci * DG + ch * 128 + 128],
                        g_sb[:, ci * T + qcw * 512 : ci * T + qcw * 512 + 512],
                        start=(ci == 0),
                        stop=(ci == CI - 1),
                    )
                ot = outp.tile([128, 512], F32, tag="out", name=f"ot{qcw}{ch}")
                nc.vector.tensor_copy(ot[:], wp[:])
                nc.sync.dma_start(
                    outT_ext[ch * 128 : (ch + 1) * 128, qcw * 512 : (qcw + 1) * 512],
                    ot[:],
                )

        def make_norm(qc, hp, av, rr):
            # deferred into the next unit's kb loop so the chain never
            # head-of-line-blocks the PE queue (recip already ran at unit end)
            def run():
                bs0 = attp.tile([64, 512], F32, tag="bs0", name=f"bs0_{qc}{hp}")
                bs1 = attp.tile([64, 512], F32, tag="bs1", name=f"bs1_{qc}{hp}")
                nc.gpsimd.partition_broadcast(bs0[:], rr[0:1, 0:512])
                nc.gpsimd.partition_broadcast(bs1[:], rr[0:1, 512:1024])
                at0 = attp.tile([64, 512], F16, tag="at0", name=f"at0_{qc}{hp}")
                at1 = attp.tile([64, 512], F16, tag="at1", name=f"at1_{qc}{hp}")
                nc.vector.tensor_tensor(
                    at0[:], av[0:D, 0:512], bs0[:], AluOpType.mult
                )
                nc.vector.tensor_tensor(
                    at1[:], av[0:D, 512:1024], bs1[:], AluOpType.mult
                )
                if qc < QC - 1:
                    # chunks 0..2: one AllGather per chunk (both head-pairs)
                    if hp == 0:
                        state[("ad", qc)] = dram.tile(
                            [256, 512], F16, tag=f"ad{qc}", name=f"ad{qc}"
                        )
                    attn_dram = state[("ad", qc)]
                    ofs = hp * 128
                else:
                    # last chunk: per-unit gathers so the tail only waits on
                    # the final head-pair's small gather
                    attn_dram = dram.tile(
                        [128, 512], F16, tag=f"ad{qc}_{hp}", name=f"ad{qc}_{hp}"
                    )
                    ofs = 0
                # partition shift for the odd head happens here for free
                nc.sync.dma_start(attn_dram[ofs : ofs + 64, :], at0[:])
                nc.sync.dma_start(attn_dram[ofs + 64 : ofs + 128, :], at1[:])
                if DBG:
                    sl_ = slice(hp * T + qc * 512, hp * T + qc * 512 + 512)
                    nc.sync.dma_start(dbga_ext[0:64, sl_], at0[:])
                    nc.sync.dma_start(dbga_ext[64:128, sl_], at1[:])
                    u_ = 2 * qc + hp
                    nc.sync.dma_start(
                        dbgr_ext[0:1, u_ * 1024 : (u_ + 1) * 1024], rr[:]
                    )
                if qc < QC - 1:
                    if hp == 1:
                        gathered = dram.tile(
                            [HG * 256, 512], F16, tag=f"gd{qc}", name=f"gd{qc}"
                        )
                        nc.gpsimd.collective_compute(
                            "AllGather",
                            mybir.AluOpType.bypass,
                            replica_groups=REPLICA_GROUPS,
                            ins=[attn_dram[:].opt()],
                            outs=[gathered[:].opt()],
                        )
                        nc.sync.dma_start(
                            g_sb[:, :].rearrange("p (ci t) -> p ci t", ci=CI)[
                                :, :, qc * 512 : qc * 512 + 512
                            ],
                            gathered[:].rearrange("(ci p) t -> p ci t", p=128),
                        )
                        wo_stage.append(qc)
                else:
                    gathered = dram.tile(
                        [HG * 128, 512], F16, tag=f"gd{qc}_{hp}", name=f"gd{qc}_{hp}"
                    )
                    nc.gpsimd.collective_compute(
                        "AllGather",
                        mybir.AluOpType.bypass,
                        replica_groups=REPLICA_GROUPS,
                        ins=[attn_dram[:].opt()],
                        outs=[gathered[:].opt()],
                    )
                    nc.sync.dma_start(
                        g_sb[:, :].rearrange("p (g c) -> p g c", g=HG)[
                            :, :, hp * T + qc * 512 : hp * T + qc * 512 + 512
                        ],
                        gathered[:].rearrange("(g p) t -> p g t", p=128),
                    )
                    if hp == 1:
                        wo_stage.append(qc)

            return run

        def attention_unit(qc, hp):
            nkb = 4 * (qc + 1)
            h0, h1 = 2 * hp, 2 * hp + 1
            q0 = qT_sb[0:64, hp * T + qc * 512 : hp * T + qc * 512 + 512]
            q1 = qT_sb[64:128, hp * T + qc * 512 : hp * T + qc * 512 + 512]
            av = av_ps.tile([VROW, 1024], F32, tag="av", name="av")
            ets = {}

            def emit_qk(kb):
                r = kb - 4 * qc
                c0 = 128 * r if r > 0 else 0
                # both heads' scoresT in one 2-bank tile; the two matmuls
                # ride different PE row groups and run concurrently
                qkp = qk_ps.tile([128, 1024], F32, tag="qk", name="qkp")
                nc.tensor.matmul(
                    qkp[:, c0:512],
                    kT_sb[0:64, hp * T + kb * 128 : hp * T + kb * 128 + 128],
                    q0[:, c0:512],
                    start=True,
                    stop=True,
                )
                nc.tensor.matmul(
                    qkp[:, 512 + c0 : 1024],
                    kT_sb[64:128, hp * T + kb * 128 : hp * T + kb * 128 + 128],
                    q1[:, c0:512],
                    start=True,
                    stop=True,
                )
                et = expp.tile([128, 1024], F16, tag="exp", name="e")
                ets[kb] = et
                # single bias-free exp over both heads (ALiBi lives in the V
                # prescale); the [0:c0) strips are stale-but-finite junk that
                # AV never reads
                nc.scalar.activation(
                    et[:], qkp[:], ACT.Exp, scale=float(D) ** -0.5
                )
                if r >= 0:
                    for eoff in (0, 512):
                        nc.vector.tensor_tensor(
                            et[:, eoff + c0 : eoff + c0 + 128],
                            et[:, eoff + c0 : eoff + c0 + 128],
                            tri_sb[:],
                            AluOpType.mult,
                        )
                if DBG and qc == 0 and hp == 0:
                    nc.sync.dma_start(
                        dbge_ext[:, kb * 1024 : (kb + 1) * 1024], et[:]
                    )

            def emit_av(kb):
                r = kb - 4 * qc
                c0 = 128 * r if r > 0 else 0
                et = ets.pop(kb)
                for h, eoff in ((h0, 0), (h1, 512)):
                    nc.tensor.matmul(
                        av[:, eoff + c0 : eoff + 512],
                        v_sb[
                            :,
                            kb * HL * VROW + h * VROW : kb * HL * VROW
                            + (h + 1) * VROW,
                        ],
                        et[:, eoff + c0 : eoff + 512],
                        start=(kb == 0),
                        stop=(kb == nkb - 1),
                    )

            # kb processed in pairs: 4 QK matmuls (64x128 tiling mode) then
            # 4 AV matmuls (128x128 mode) per step, halving PE mode switches
            for kb2 in range(0, nkb + LAG, 2):
                if kb2 == 2 and state["pending_norm"] is not None:
                    state["pending_norm"]()
                    state["pending_norm"] = None
                for kb in (kb2, kb2 + 1):
                    if kb < nkb:
                        emit_qk(kb)
                for kb in (kb2, kb2 + 1):
                    if LAG <= kb < nkb + LAG:
                        emit_av(kb - LAG)

            # reciprocal of both heads' denominator rows right away (DVE is
            # free here); the rest of the norm chain is deferred
            dn = nrm.tile([VROW, 1024], F32, tag="dn", name=f"dn{qc}{hp}")
            nc.vector.tensor_copy(dn[D : D + 1, :], av[D : D + 1, 0:1024])
            rri = nrm.tile([1, 1024], F32, tag="rri", name=f"rri{qc}{hp}")
            nc.sync.dma_start(rri[:], dn[D : D + 1, :])
            rr = nrm.tile([1, 1024], F32, tag="rr", name=f"rr{qc}{hp}")
            nc.vector.reciprocal_approx_fast(rr[:], rri[:])
            if DBG:
                u_ = 2 * qc + hp
                avc = nrm.tile([VROW, 1024], F32, tag="avc", name=f"avc{qc}{hp}")
                nc.vector.tensor_copy(avc[:], av[:])
                nc.sync.dma_start(
                    dbgav_ext[:, u_ * 1024 : (u_ + 1) * 1024], avc[:]
                )

            # out-projections whose gathers fired >= 3 units ago
            while wo_queue:
                emit_wo_chunk(wo_queue.pop(0))
            wo_queue.extend(wo_stage2)
            del wo_stage2[:]
            wo_stage2.extend(wo_stage)
            del wo_stage[:]

            state["pending_norm"] = make_norm(qc, hp, av, rr)

        # ---------------- the pipeline -----------------------------------
        for qc in range(QC):
            proj_qk(wk_sb, kT_sb, bk_sb, qc, "k")
            proj_v(qc)
            proj_qk(wq_sb, qT_sb, bq_sb, qc, "q")
            for hp in (0, 1):
                attention_unit(qc, hp)

        # final norm chain first (its DVE ops run uncontended), then the
        # remaining out-projections fill the PE while the last gather runs
        state["pending_norm"]()
        for qcw in wo_queue + wo_stage2 + wo_stage:
            emit_wo_chunk(qcw)

        if DBG:
            nc.sync.dma_start(dbgg_ext[:], g_sb[:])
            nc.sync.dma_start(dbgqk_ext[:, 0 : 2 * T], qT_sb[:])
            nc.sync.dma_start(dbgqk_ext[:, 2 * T : 4 * T], kT_sb[:])
            nc.sync.dma_start(dbgv_ext[:], v_sb[:])

    nc.compile()
    return nc


def _get_compiled():
    global _COMPILED
    if _COMPILED is None:
        _COMPILED = _build()
    return _COMPILED


def _make_in_maps(x, Wq, bq, Wk, bk, Wv, bv, Wo, bo):
    slopes = _alibi_slopes(H)
    # tri[p, f] = 1 where key-offset p <= q-offset f (causal keep region)
    tri = np.triu(np.ones((128, 128), dtype=np.float16))
    ones1 = np.ones((1, 128), dtype=np.float32)
    in_maps = []
    for c in range(N_CORES):
        b, hg = divmod(c, HG)
        sl = slice(hg * DG, (hg + 1) * DG)
        # expb[p, h*TB+tb] = exp(-slope_h * j - ln 64) at key j = 128*tb + p
        expb = np.empty((128, HL * TB), dtype=np.float32)
        p = np.arange(128, dtype=np.float64)[:, None]
        for h in range(HL):
            s = float(slopes[hg * HL + h])
            tbs = np.arange(TB, dtype=np.float64)[None, :]
            expb[:, h * TB : (h + 1) * TB] = np.exp(
                -s * (128.0 * tbs + p) - LN_SHIFT
            ).astype(np.float32)
        in_maps.append(
            {
                "xT": np.ascontiguousarray(x[b].T).astype(np.float16),
                "wq": np.ascontiguousarray(Wq[:, sl]).astype(np.float16),
                "wk": np.ascontiguousarray(Wk[:, sl]).astype(np.float16),
                "wv": np.ascontiguousarray(Wv[:, sl]).astype(np.float16),
                "wo": np.ascontiguousarray(Wo[:, sl]).astype(np.float16),
                "bq2": np.ascontiguousarray(bq[sl].reshape(2, 128)).astype(np.float32),
                "bk2": np.ascontiguousarray(bk[sl].reshape(2, 128)).astype(np.float32),
                "expb": expb,
                "tri": tri,
                "ones1": ones1,
            }
        )
    return in_maps


def kernel(x, Wq, bq, Wk, bk, Wv, bv, Wo, bo):
    global last_exec_time_ns, last_trace_path
    x = np.asarray(x, dtype=np.float32)
    Wq = np.asarray(Wq, dtype=np.float32)
    bq = np.asarray(bq, dtype=np.float32)
    Wk = np.asarray(Wk, dtype=np.float32)
    bk = np.asarray(bk, dtype=np.float32)
    Wv = np.asarray(Wv, dtype=np.float32)
    bv = np.asarray(bv, dtype=np.float32)
    Wo = np.asarray(Wo, dtype=np.float32)
    bo = np.asarray(bo, dtype=np.float32)

    from concourse import bass_utils

    trace = bool(os.environ.get("BASS_KERNEL_TRACE"))
    kwargs = {}
    if trace:
        try:
            import sys
            import types

            import antenv

            if "antenv.axon_hooks" not in sys.modules:
                hooks = types.ModuleType("antenv.axon_hooks")
                _h = [None]
                hooks.set_axon_ntff_profile_hook = lambda fn: _h.__setitem__(0, fn)
                hooks.get_axon_ntff_profile_hook = lambda: _h[0]
                sys.modules["antenv.axon_hooks"] = hooks
                antenv.axon_hooks = hooks
                from trn_agent_boot.trn_boot import _ntff_profile_via_ctypes

                hooks.set_axon_ntff_profile_hook(
                    _ntff_profile_via_ctypes("/opt/axon/libaxon_pjrt.so")
                )
            bass_utils.upload_artifacts = lambda tmpdir: "local://" + str(tmpdir)
            kwargs = {"trace": True, "tmpdir": os.environ.get("BASS_KERNEL_TRACE_DIR")}
        except Exception as e:  # pragma: no cover
            print(f"trace setup failed ({e}); running untraced")
            trace = False

    nc = _get_compiled()
    in_maps = _make_in_maps(x, Wq, bq, Wk, bk, Wv, bv, Wo, bo)
    res = bass_utils.run_bass_kernel_spmd(
        nc, in_maps, core_ids=list(range(N_CORES)), **kwargs
    )
    if trace:
        last_exec_time_ns = res.exec_time_ns
        if res.instructions_and_trace is not None:
            last_trace_path = res.instructions_and_trace[1]

    # final-projection bias (incl. the v bias folded through Wo) on host
    bfin = bv @ Wo + bo  # [C]
    out = np.empty((B, T, C), dtype=np.float32)
    for c in range(N_CORES):
        b, hg = divmod(c, HG)
        sl = slice(hg * DG, (hg + 1) * DG)
        out[b, :, sl] = res.results[c]["outT"].T + bfin[sl]
    return out
